# revision 2
# baseline (speedup 1.0000x reference)
"""Trainium2 Bass kernel for nn_NodeEmbedder: embedding lookup + 2-layer LSTM,
returning the final top-layer hidden state.

Sharding: data-parallel over the node axis, 8 cores x 512 nodes; embedding
table and LSTM weights replicated. Inside each core the 512 nodes are split
into 2 independent streams of 256 for engine pipelining.

Layout: the recurrence runs in "transposed" orientation (z.T = W.T @ x.T), so
LSTM weights serve directly as the stationary matmul operand and hidden/cell
states flow as [hidden, node] tiles with no per-step transposes. The embedding
gather uses dma_gather(transpose=True) on a host-prepared bf16 hi/lo split of
the table; hi+lo are re-merged on-chip to recover fp32 token vectors. All
matmuls run as float32r (fp22 multiplies, fp32 accumulate).
"""

import sys

for _p in ("/opt/trn_rl_repo", "/root/.axon_site/_ro/trn_rl_repo"):
    if _p not in sys.path:
        sys.path.insert(0, _p)

import numpy as np
import ml_dtypes

import concourse.bass as bass
import concourse.mybir as mybir
import concourse.tile as tile
from concourse import bacc
from concourse.bass import ts
from concourse.bass_utils import run_bass_kernel_spmd

P = 128
N_CORES = 8
F32 = mybir.dt.float32
F32R = mybir.dt.float32r
BF16 = mybir.dt.bfloat16
I16 = mybir.dt.int16
SIG = mybir.ActivationFunctionType.Sigmoid
TANH = mybir.ActivationFunctionType.Tanh
ADD = mybir.AluOpType.add
MULT = mybir.AluOpType.mult


def build(T=64, V=30000, H=256, B=512, L=2, bias_nonzero=False):
    """Build + compile the per-core program. B nodes per core, 2 streams."""
    KC = H // P          # K chunks (2)
    MT = 4 * H // P      # M tiles (8)
    S = 2                # streams
    BH = B // S          # nodes per stream (256)
    IXC = B // 16        # index columns per step (32)

    nc = bacc.Bacc("TRN2", target_bir_lowering=False, debug=False)
    eh_d = nc.dram_tensor("eh", [V, H], BF16, kind="ExternalInput").ap()
    el_d = nc.dram_tensor("el", [V, H], BF16, kind="ExternalInput").ap()
    wx_d = nc.dram_tensor("wx", [L, KC, P, 4 * H], F32R, kind="ExternalInput").ap()
    wh_d = nc.dram_tensor("wh", [L, KC, P, 4 * H], F32R, kind="ExternalInput").ap()
    ix_d = nc.dram_tensor("ix", [P, T, IXC], I16, kind="ExternalInput").ap()
    id_d = nc.dram_tensor("ident", [P, P], F32R, kind="ExternalInput").ap()
    bv_d = None
    if bias_nonzero:
        bv_d = nc.dram_tensor("bv", [P, L, MT], F32, kind="ExternalInput").ap()
    out_d = nc.dram_tensor("out", [B, H], F32, kind="ExternalOutput").ap()

    def r(ap):
        return ap.bitcast(F32R)

    with tile.TileContext(nc) as tc:
        with (
            tc.tile_pool(name="consts", bufs=1) as consts,
            tc.tile_pool(name="gat", bufs=4) as gat,
            tc.tile_pool(name="tok", bufs=4) as tokp,
            tc.tile_pool(name="zp", bufs=2, space="PSUM") as zp,
            tc.tile_pool(name="gates", bufs=3) as gp,
            tc.tile_pool(name="tmp", bufs=3) as tp,
            tc.tile_pool(name="state", bufs=2) as sp,
            tc.tile_pool(name="outp", bufs=1) as outp,
        ):
            wsb = [[None] * 2 for _ in range(L)]
            for l in range(L):
                for mi, wd in enumerate((wx_d, wh_d)):
                    w = consts.tile([P, KC, 4 * H], F32R, name=f"w{mi}{l}")
                    nc.sync.dma_start(out=w[:], in_=wd[l].rearrange("k p m -> p k m"))
                    wsb[l][mi] = w
            idx = consts.tile([P, T, IXC], I16)
            nc.sync.dma_start(out=idx[:], in_=ix_d[:])
            ident = consts.tile([P, P], F32R)
            nc.sync.dma_start(out=ident[:], in_=id_d[:])
            bv = None
            if bias_nonzero:
                bv = consts.tile([P, L, MT], F32)
                nc.sync.dma_start(out=bv[:], in_=bv_d[:])

            h_cur = [[None] * S for _ in range(L)]
            c_cur = [[None] * S for _ in range(L)]

            # gate order after host-side permutation of W columns: i, f, o, g
            # m-tiles: i=(0,1) f=(2,3) o=(4,5) g=(6,7); one PSUM bank per gate
            for t in range(T):
                gh = gat.tile([P, KC, B], BF16, tag="gh", name=f"gh{t}")
                gl = gat.tile([P, KC, B], BF16, tag="gl", name=f"gl{t}")
                for g_t, e_d in ((gh, eh_d), (gl, el_d)):
                    nc.gpsimd.dma_gather(
                        out_ap=g_t[:],
                        in_ap=e_d[:],
                        idxs_ap=idx[:, t, :],
                        num_idxs=B,
                        num_idxs_reg=B,
                        elem_size=H,
                        transpose=True,
                    )
                tokt = tokp.tile([P, KC, B], F32R, tag="tok", name=f"tok{t}")
                nc.gpsimd.tensor_tensor(out=tokt[:], in0=gh[:], in1=gl[:], op=ADD)

                for s in range(S):
                    for l in range(L):
                        if l == 0:
                            xr = [tokt[:, kc, ts(s, BH)] for kc in range(KC)]
                        else:
                            xr = [h_cur[0][s][:, kc, :] for kc in range(KC)]
                        hr = None
                        if t > 0:
                            hr = [h_cur[l][s][:, kc, :] for kc in range(KC)]

                        z = zp.tile([P, MT, BH], F32, tag="z", name=f"z{t}{s}{l}")
                        for m in (6, 7, 0, 1, 2, 3, 4, 5):
                            ops = [(wsb[l][0], xr)]
                            if hr is not None:
                                ops.append((wsb[l][1], hr))
                            n_acc = len(ops) * KC
                            k = 0
                            for w, rhs in ops:
                                for kc in range(KC):
                                    nc.tensor.matmul(
                                        out=z[:, m, :],
                                        lhsT=w[:, kc, ts(m, P)],
                                        rhs=rhs[kc],
                                        start=(k == 0),
                                        stop=(k == n_acc - 1),
                                    )
                                    k += 1
                            if bias_nonzero:
                                nc.vector.tensor_scalar_add(
                                    out=z[:, m, :],
                                    in0=z[:, m, :],
                                    scalar1=bv[:, l, m : m + 1],
                                )

                        gg = gp.tile([P, 2, BH], F32, tag="gg", name=f"gg{t}{s}{l}")
                        nc.scalar.activation(out=gg[:], in_=z[:, 6:8, :], func=TANH)
                        gi = gp.tile([P, 2, BH], F32, tag="gi", name=f"gi{t}{s}{l}")
                        nc.scalar.activation(out=gi[:], in_=z[:, 0:2, :], func=SIG)
                        gf = gp.tile([P, 2, BH], F32, tag="gf", name=f"gf{t}{s}{l}")
                        nc.scalar.activation(out=gf[:], in_=z[:, 2:4, :], func=SIG)
                        go = gp.tile([P, 2, BH], F32, tag="go", name=f"go{t}{s}{l}")
                        nc.scalar.activation(out=go[:], in_=z[:, 4:6, :], func=SIG)

                        c_new = sp.tile(
                            [P, KC, BH], F32, tag=f"c{l}{s}", name=f"c{t}{s}{l}"
                        )
                        if t == 0:
                            nc.vector.tensor_tensor(
                                out=c_new[:], in0=gi[:], in1=gg[:], op=MULT
                            )
                        else:
                            ig = tp.tile([P, KC, BH], F32, tag="ig", name=f"ig{t}{s}{l}")
                            nc.vector.tensor_tensor(
                                out=ig[:], in0=gi[:], in1=gg[:], op=MULT
                            )
                            nc.vector.tensor_tensor(
                                out=c_new[:], in0=gf[:], in1=c_cur[l][s][:], op=MULT
                            )
                            nc.vector.tensor_tensor(
                                out=c_new[:], in0=c_new[:], in1=ig[:], op=ADD
                            )
                        tc_t = tp.tile([P, KC, BH], F32, tag="tc", name=f"tc{t}{s}{l}")
                        nc.scalar.activation(out=tc_t[:], in_=c_new[:], func=TANH)
                        h_new = sp.tile(
                            [P, KC, BH], F32R, tag=f"h{l}{s}", name=f"h{t}{s}{l}"
                        )
                        nc.vector.tensor_tensor(
                            out=h_new[:], in0=go[:], in1=tc_t[:], op=MULT
                        )
                        c_cur[l][s] = c_new
                        h_cur[l][s] = h_new

            # epilogue: transpose h_cur[L-1] back to [node, hidden] and store
            osb = outp.tile([P, B // P, H], F32)
            trp = zp.tile([P, MT, BH], F32, tag="z", name="trp")
            for s in range(S):
                for gl_i in range(BH // P):
                    for kc in range(KC):
                        j = (s * (BH // P) + gl_i) * KC + kc
                        nc.tensor.transpose(
                            out=r(trp[:, j, :P]),
                            in_=h_cur[L - 1][s][:, kc, ts(gl_i, P)],
                            identity=ident[:],
                        )
                        nc.vector.tensor_copy(
                            out=osb[:, s * (BH // P) + gl_i, ts(kc, P)],
                            in_=trp[:, j, :P],
                        )
            nc.sync.dma_start(
                out=out_d.rearrange("(g p) h -> p g h", p=P), in_=osb[:]
            )

    nc.compile()
    return nc


def prep_inputs(data, embed, Wx, Wh, b, T, V, H, B):
    """Host-side input prep: gate permutation, bf16 hi/lo split, index wrap."""
    L = Wx.shape[0]
    KC = H // P
    data = np.asarray(data).astype(np.int32)
    embed = np.asarray(embed, dtype=np.float32)
    Wx = np.asarray(Wx, dtype=np.float32)
    Wh = np.asarray(Wh, dtype=np.float32)
    b = np.asarray(b, dtype=np.float32)

    # reference gate order i,f,g,o -> kernel order i,f,o,g
    perm = np.concatenate(
        [np.arange(0, H), np.arange(H, 2 * H), np.arange(3 * H, 4 * H),
         np.arange(2 * H, 3 * H)]
    )
    wx_p = Wx[:, :, perm].reshape(L, KC, P, 4 * H)
    wh_p = Wh[:, :, perm].reshape(L, KC, P, 4 * H)
    b_p = b[:, perm]
    bias_nonzero = bool(np.any(b_p))
    # bv[p, l, m] = b_p[l, m*128 + p]
    bv = np.transpose(b_p.reshape(L, 4 * H // P, P), (2, 0, 1)).copy()

    eh = embed.astype(ml_dtypes.bfloat16)
    el = (embed - eh.astype(np.float32)).astype(ml_dtypes.bfloat16)

    ident = np.eye(P, dtype=np.float32)

    per_core = []
    for c in range(N_CORES):
        d = data[c * B : (c + 1) * B]  # [B, T]
        # wrapped indices: token j of step t at (partition j%16, col j//16),
        # replicated across the 8 gpsimd cores (partition groups of 16)
        ixw = np.zeros((P, T, B // 16), np.int16)
        for t in range(T):
            w16 = d[:, t].reshape(B // 16, 16).T.astype(np.int16)  # [16, B/16]
            ixw[:, t, :] = np.tile(w16, (8, 1))
        m = {"eh": eh, "el": el, "wx": wx_p, "wh": wh_p, "ix": ixw, "ident": ident}
        if bias_nonzero:
            m["bv"] = bv
        per_core.append(m)
    return per_core, bias_nonzero


_CACHE = {}


def kernel(data, embed, Wx, Wh, b):
    data = np.asarray(data)
    embed = np.asarray(embed)
    N, T = data.shape
    V, H = embed.shape
    B = N // N_CORES
    per_core, bias_nonzero = prep_inputs(data, embed, Wx, Wh, b, T, V, H, B)
    key = (T, V, H, B, bias_nonzero)
    if key not in _CACHE:
        _CACHE[key] = build(T=T, V=V, H=H, B=B, L=Wx.shape[0],
                            bias_nonzero=bias_nonzero)
    nc = _CACHE[key]
    res = run_bass_kernel_spmd(nc, per_core, list(range(N_CORES)))
    out = np.concatenate([r["out"] for r in res.results], axis=0)
    return out.astype(np.float32)
